# revision 28
# baseline (speedup 1.0000x reference)
"""Trainium2 Bass kernel for nn_AstroSymbolicEpisodicLayer.

Strategy
--------
8 cores = (batch b in 0..3) x (key-half h in 0..1).  Each core computes the
full 2048-query attention against its 1024-key half; the pair combines
partial (unnormalized) outputs and score row-sums with pair-local
AllReduces pipelined per query chunk, then finalizes the full output (the
host reads one core per batch).  One SPMD program; per-core differences are
pure data.

Math decomposition (validated to ~1e-6 in fp32 vs the jax reference):
 - Circular-convolution binding via a packed real DFT implemented as
   1024x1024 matmuls.  Packed layout: rows 0..511 are Re bins 0..511, row 512
   is Re bin 512, rows 513..1023 are Im bins 1..511.  Pointwise complex
   multiply pairs row f with row 512+f (tile j with tile j+4), plus a 2-row
   fixup for the purely-real bins 0 and 512.
 - rfft(role_vecs) = role_weights @ rfft(role_norm) by linearity; the softmax
   denominator cancels in the K_bound l2-normalization so we use plain exp.
 - Host weight foldings remove two full projection stages: K is never
   materialized (Kf = kT.T @ (Wk F) + bk F and hidden = relu(kT.T @ (Wk Wr1)
   + bk Wr1 + br1)), and the output projection folds into V (VW = vT.T @
   (Wv Wo); the bv term becomes a constant output bias because the attention
   rows are normalized).
 - Per-token normalizations are folded: 1/||K_bound|| becomes a per-partition
   scalar in the scores epilogue (scores computed keys-major), 1/||Q|| scales
   q_n in SBUF, and the attention row-sum is folded into the output epilogue
   (after the pair AllReduce).

Everything is feature-major ("xT" = [feature, token]) so every chained matmul
is out = lhsT.T @ rhs with contraction on the partition dim.
"""

import sys

for _p in ("/opt/trn_rl_repo", "/root/.axon_site/_ro/trn_rl_repo"):
    if _p not in sys.path:
        sys.path.append(_p)

import numpy as np
import ml_dtypes

import concourse.bass as bass
import concourse.bacc as bacc
import concourse.tile as tile
from concourse import mybir
from concourse.bass_utils import run_bass_kernel_spmd

BF16 = ml_dtypes.bfloat16
F32 = mybir.dt.float32
BF = mybir.dt.bfloat16

B, TQ, TK, D, R, H = 4, 2048, 2048, 1024, 64, 512
TAU_BASE, ASTRO_DECAY = 1.0, 0.95
P = 128
NJ = D // P            # 8 feature chunks
TK2 = TK // 2          # keys per core (1024)
NT2 = TK2 // 512       # 2 key token chunks
NKT2 = TK2 // P        # 8 key tiles of 128
NQT = TQ // 512        # 4 query chunks of 512
TQH = TQ // 2
AF = mybir.ActivationFunctionType
ALU = mybir.AluOpType
CC_GROUPS = [[0, 1], [2, 3], [4, 5], [6, 7]]


# ---------------------------------------------------------------- host consts
def _build_dft_mats(n=1024):
    j = np.arange(n)[:, None].astype(np.float64)
    f = np.arange(n // 2)[None, :].astype(np.float64)
    F = np.zeros((n, n), np.float64)
    F[:, :512] = np.cos(2 * np.pi * j * f / n)
    F[:, 512] = np.cos(np.pi * j[:, 0])
    fi = np.arange(1, 512)[None, :].astype(np.float64)
    F[:, 513:] = -np.sin(2 * np.pi * j * fi / n)

    G = np.zeros((n, n), np.float64)
    d = np.arange(n)[None, :].astype(np.float64)
    G[0, :] = 1.0 / n
    ff = np.arange(1, 512)[:, None].astype(np.float64)
    G[1:512, :] = 2.0 * np.cos(2 * np.pi * ff * d / n) / n
    G[512, :] = ((-1.0) ** d[0]) / n
    G[513:, :] = -2.0 * np.sin(2 * np.pi * ff * d / n) / n
    return F.astype(np.float32), G.astype(np.float32)


def _pack_rfft(x):
    X = np.fft.rfft(np.asarray(x, np.float64), axis=-1)
    out = np.empty(x.shape, np.float32)
    out[..., :512] = X.real[..., :512]
    out[..., 512] = X.real[..., 512]
    out[..., 513:] = X.imag[..., 1:512]
    return out


# ---------------------------------------------------------------- bass kernel
def _emit(nc: bass.Bass):
    # ---- I/O declarations (names = in_map keys) ----
    din = {}
    for nm, shp, dt in [
        ("qT", [D, TQ], BF), ("kT", [D, TK2], BF), ("vT", [D, TK2], BF),
        ("wq", [D, D], BF), ("wkf", [D, D], BF), ("gmat", [D, D], BF),
        ("vwo", [D, D], BF), ("wkr1", [D, H], BF), ("wr2", [H, R], BF),
        ("pf", [R, D], BF),
        ("bq", [P, NJ], F32), ("bkf", [P, NJ], F32), ("bvo", [P, NJ], F32),
        ("brk", [P, H // P], F32), ("scal", [1, 2], F32),
    ]:
        din[nm] = nc.declare_dram_parameter(nm, shp, dt, isOutput=False)
    outT = nc.declare_dram_parameter("outT", [D, TQ], F32, isOutput=True)
    ns_out = nc.declare_dram_parameter("ns_out", [1, 1], F32, isOutput=True)

    # internal DRAM: broadcast staging + collective buffers (per query chunk)
    rk_stage = nc.dram_tensor("rk_stage", [1, TK2], F32)
    rq_stage = nc.dram_tensor("rq_stage", [1, TQ], BF)
    rsf_stage = nc.dram_tensor("rsf_stage", [1, TQ], F32)
    c_stage = nc.dram_tensor("c_stage", [1, 1], F32)
    po_in = [nc.dram_tensor(f"po_in{qc}", [D, 512], BF) for qc in range(NQT)]
    po_out = [nc.dram_tensor(f"po_out{qc}", [D, 512], BF) for qc in range(NQT)]
    rs_in = nc.dram_tensor("rs_in", [1, TQ], F32)
    rs_out = nc.dram_tensor("rs_out", [1, TQ], F32)

    c3 = lambda t: t.rearrange("(po pi) x -> pi po x", pi=P)

    with tile.TileContext(nc) as tc, \
         tc.tile_pool(name="w", bufs=2) as wpool, \
         tc.tile_pool(name="wsmall", bufs=1) as wsmall, \
         tc.tile_pool(name="kinvw", bufs=1) as kinvw, \
         tc.tile_pool(name="qstream", bufs=2) as qstream, \
         tc.tile_pool(name="outstage", bufs=2) as outstage, \
         tc.tile_pool(name="persist", bufs=1) as persist, \
         tc.tile_pool(name="kfc", bufs=2) as kfcpool, \
         tc.tile_pool(name="scores", bufs=2) as scorepool, \
         tc.tile_pool(name="attst", bufs=3) as attst, \
         tc.tile_pool(name="tmp", bufs=2) as tmp, \
         tc.tile_pool(name="cmt", bufs=2) as cmt, \
         tc.tile_pool(name="small", bufs=1) as small, \
         tc.tile_pool(name="rows", bufs=1) as rows, \
         tc.tile_pool(name="ps", bufs=4, space="PSUM") as ps, \
         tc.tile_pool(name="psrow", bufs=3, space="PSUM") as psrow:

        # ---- constants ----
        ones_k = small.tile([P, 1], BF, tag="ones_k")      # lhsT for partition sums
        nc.vector.memset(ones_k, 1.0)
        bq_sb = small.tile([P, NJ], F32, tag="bq")
        nc.sync.dma_start(bq_sb, din["bq"][:])
        bkf_sb = small.tile([P, NJ], F32, tag="bkf")
        nc.sync.dma_start(bkf_sb, din["bkf"][:])
        bvo_sb = small.tile([P, NJ], F32, tag="bvo")
        nc.sync.dma_start(bvo_sb, din["bvo"][:])
        brk_sb = small.tile([P, H // P], F32, tag="brk")
        nc.sync.dma_start(brk_sb, din["brk"][:])
        scal_sb = small.tile([1, 2], F32, tag="scal")
        nc.sync.dma_start(scal_sb, din["scal"][:])

        # pre-touch const tiles (ACT has a small hardware wait table)
        pre_s = small.tile([P, 1], F32, tag="pre_s")
        nc.scalar.activation(pre_s, bq_sb[:, 0:1], AF.Identity, bias=bkf_sb[:, 0:1])
        nc.scalar.activation(pre_s, pre_s, AF.Relu, bias=brk_sb[:, 0:1])
        pre_v = small.tile([P, 1], F32, tag="pre_v")
        nc.vector.tensor_scalar_add(pre_v, pre_s, bvo_sb[:, 0:1])
        pre_v2 = small.tile([1, 1], F32, tag="pre_v2")
        nc.vector.tensor_scalar_add(pre_v2, scal_sb[0:1, 0:1], 0.0)

        def load_w(name, shape3):
            t = wpool.tile(shape3, BF, tag="w", name=name)
            nc.gpsimd.dma_start(t, c3(din[name][:]))
            return t

        # kT half resident (used by both the hidden pass and the Kf pass)
        kin = kinvw.tile([P, NJ, TK2], BF, tag="kinvw", name="kin")
        nc.sync.dma_start(kin, c3(din["kT"][:]))

        # ==================== S_A: hidden = relu(WkR1.T @ kT + brk)
        wkr1 = wsmall.tile([P, NJ, H], BF, tag="wkr1")
        nc.gpsimd.dma_start(wkr1, c3(din["wkr1"][:]))
        hidden = scorepool.tile([P, H // P, TK2], BF, tag="scT", name="hidden")
        for t in range(NT2):
            for jh in range(H // P):
                pst = ps.tile([P, 512], F32, tag="mm")
                for i in range(NJ):
                    nc.tensor.matmul(pst, wkr1[:, i, jh * P:(jh + 1) * P],
                                     kin[:, i, t * 512:(t + 1) * 512],
                                     start=(i == 0), stop=(i == NJ - 1))
                nc.scalar.activation(hidden[:, jh, t * 512:(t + 1) * 512], pst,
                                     AF.Relu, bias=brk_sb[:, jh:jh + 1])

        # ==================== S_C: role logits -> exp -> Rf (SBUF resident)
        wr2 = wsmall.tile([P, H // P, R], BF, tag="wr2")
        nc.gpsimd.dma_start(wr2, c3(din["wr2"][:]))
        rw = persist.tile([R, TK2], BF, tag="rw")
        for t in range(NT2):
            ps64 = ps.tile([R, 512], F32, tag="mm")
            for i2 in range(H // P):
                nc.tensor.matmul(ps64, wr2[:, i2, :], hidden[:, i2, t * 512:(t + 1) * 512],
                                 start=(i2 == 0), stop=(i2 == H // P - 1))
            nc.scalar.activation(rw[:, t * 512:(t + 1) * 512], ps64, AF.Exp)
        pf = wsmall.tile([R, D], BF, tag="pf")
        nc.gpsimd.dma_start(pf, din["pf"][:])
        Rf = persist.tile([P, NJ, TK2], BF, tag="big1", name="Rf")
        for t in range(NT2):
            for j in range(NJ):
                pst = ps.tile([P, 512], F32, tag="mm")
                nc.tensor.matmul(pst, pf[:, j * P:(j + 1) * P],
                                 rw[:, t * 512:(t + 1) * 512], start=True, stop=True)
                nc.scalar.activation(Rf[:, j, t * 512:(t + 1) * 512], pst, AF.Copy)

        # ==================== S_B: fused Kf -> cmul -> KB per key chunk
        wkf = load_w("wkf", [P, NJ, D])
        gmat = load_w("gmat", [P, NJ, D])
        kn = persist.tile([P, NJ, TK2], BF, tag="kn")
        kfc = [None] * NT2

        def emit_kf(t):
            kfc[t] = kfcpool.tile([P, NJ, 512], BF, tag="kfc", name=f"kfc{t}")
            for j in range(NJ):
                pst = ps.tile([P, 512], F32, tag="mm")
                for i in range(NJ):
                    nc.tensor.matmul(pst, wkf[:, i, j * P:(j + 1) * P],
                                     kin[:, i, t * 512:(t + 1) * 512],
                                     start=(i == 0), stop=(i == NJ - 1))
                nc.scalar.activation(kfc[t][:, j, :], pst, AF.Identity,
                                     bias=bkf_sb[:, j:j + 1])
            # cmul in place: kf chunk becomes Z chunk (DVE)
            kf = kfc[t]
            rfs = Rf[:, :, t * 512:(t + 1) * 512]
            zf0 = cmt.tile([1, 512], BF, tag="zf0")
            zf4 = cmt.tile([1, 512], BF, tag="zf4")
            nc.vector.tensor_mul(zf0, kf[0:1, 0, :], rfs[0:1, 0, :])
            nc.vector.tensor_mul(zf4, kf[0:1, 4, :], rfs[0:1, 4, :])
            for j in range(4):
                t2 = cmt.tile([P, 512], BF, tag="cm_t2")
                t3 = cmt.tile([P, 512], BF, tag="cm_t3")
                nc.vector.tensor_mul(t2, kf[:, j + 4, :], rfs[:, j + 4, :])
                nc.vector.tensor_mul(t3, kf[:, j + 4, :], rfs[:, j, :])
                nc.vector.tensor_mul(kf[:, j + 4, :], kf[:, j, :], rfs[:, j + 4, :])
                nc.vector.tensor_tensor(kf[:, j + 4, :], kf[:, j + 4, :], t3, ALU.add)
                nc.vector.tensor_mul(kf[:, j, :], kf[:, j, :], rfs[:, j, :])
                nc.vector.tensor_tensor(kf[:, j, :], kf[:, j, :], t2, ALU.subtract)
            # rows 0 (re_0) and 512 (re_512) are purely real
            nc.vector.tensor_copy(kf[0:1, 0, :], zf0)
            nc.vector.tensor_copy(kf[0:1, 4, :], zf4)

        def emit_kb(t):
            psn = psrow.tile([1, 512], F32, tag="rowsum", name=f"psn_kb{t}")
            for j in range(NJ):
                pst = ps.tile([P, 512], F32, tag="mm")
                for i in range(NJ):
                    nc.tensor.matmul(pst, gmat[:, i, j * P:(j + 1) * P], kfc[t][:, i, :],
                                     start=(i == 0), stop=(i == NJ - 1))
                kb = kn[:, j, t * 512:(t + 1) * 512]
                nc.scalar.activation(kb, pst, AF.Copy)
                sq = tmp.tile([P, 512], BF, tag="sq")
                nc.vector.tensor_mul(sq, kb, kb)
                nc.tensor.matmul(psn, ones_k, sq, start=(j == 0), stop=(j == NJ - 1))
            # rk chunk = 1/sqrt(norm2) -> stage to DRAM for the column reload
            srow = rows.tile([1, 512], F32, tag="rowa", name="srow")
            nc.scalar.activation(srow, psn, AF.Sqrt)
            rrow = rows.tile([1, 512], F32, tag="rowb", name="rrow")
            nc.vector.reciprocal(rrow, srow)
            nc.gpsimd.dma_start(rk_stage[0:1, t * 512:(t + 1) * 512], rrow)

        emit_kf(0)
        emit_kf(1)
        emit_kb(0)
        emit_kb(1)
        # rk as [128,8] per-key-tile partition scalars, negated for the
        # (S*(-rk) + 1) epilogue
        rk_col = small.tile([P, NKT2], F32, tag="rk_col")
        nc.sync.dma_start(
            rk_col, rk_stage.rearrange("o (kt p) -> (o p) kt", p=P))
        nrk_col = small.tile([P, NKT2], F32, tag="nrk_col")
        nc.vector.tensor_scalar_mul(nrk_col, rk_col, -1.0)

        # ==================== S_E: VW = vT.T @ (WvWo), SBUF resident
        vwo = load_w("vwo", [P, NJ, D])
        VW = kinvw.tile([P, NKT2, D], BF, tag="kinvw", name="VW")
        for to in range(NKT2):
            vin = attst.tile([P, NJ, P], BF, tag="vin", name="vin")
            nc.sync.dma_start(vin, c3(din["vT"][:])[:, :, to * P:(to + 1) * P])
            for n in range(2):
                pst = ps.tile([P, 512], F32, tag="mm")
                for i in range(NJ):
                    nc.tensor.matmul(pst, vin[:, i, :], vwo[:, i, n * 512:(n + 1) * 512],
                                     start=(i == 0), stop=(i == NJ - 1))
                nc.scalar.activation(VW[:, to, n * 512:(n + 1) * 512], pst, AF.Copy)

        # ==================== S_F: Q proj (full TQ), tau, q_n
        wq = load_w("wq", [P, NJ, D])
        qn = persist.tile([P, NJ, TQ], BF, tag="big1", name="qn")
        rq_bc = small.tile([P, TQ], BF, tag="rq_bc")
        ssc = small.tile([1, NQT], F32, tag="ssc")
        for t in range(NQT):
            qin = qstream.tile([P, NJ, 512], BF, tag="qstream", name="qin")
            nc.sync.dma_start(qin, c3(din["qT"][:])[:, :, t * 512:(t + 1) * 512])
            psn = psrow.tile([1, 512], F32, tag="rowsum")
            for j in range(NJ):
                pst = ps.tile([P, 512], F32, tag="mm")
                for i in range(NJ):
                    nc.tensor.matmul(pst, wq[:, i, j * P:(j + 1) * P], qin[:, i, :],
                                     start=(i == 0), stop=(i == NJ - 1))
                qv = qn[:, j, t * 512:(t + 1) * 512]
                nc.scalar.activation(qv, pst, AF.Identity, bias=bq_sb[:, j:j + 1])
                sq = tmp.tile([P, 512], BF, tag="sq")
                nc.vector.tensor_mul(sq, qv, qv)
                nc.tensor.matmul(psn, ones_k, sq, start=(j == 0), stop=(j == NJ - 1))
            # sqrt of norms; accumulate sum(sqrt) for the surprise mean
            srow = rows.tile([1, 512], F32, tag="rowa", name="srow")
            nc.scalar.activation(srow, psn, AF.Sqrt, accum_out=ssc[0:1, t:t + 1])
            rrow = rows.tile([1, 512], F32, tag="rowb", name="rrow")
            nc.vector.reciprocal(rrow, srow)
            rrow16 = rows.tile([1, 512], BF, tag="rrow16")
            nc.vector.tensor_copy(rrow16, rrow)
            nc.gpsimd.dma_start(rq_stage[0:1, t * 512:(t + 1) * 512], rrow16)
        nc.sync.dma_start(
            rq_bc, bass.AP(tensor=rq_stage, offset=0, ap=[[0, P], [1, TQ]]))
        ss = small.tile([1, 1], F32, tag="ss")
        nc.vector.reduce_sum(ss, ssc, axis=mybir.AxisListType.X)
        # new_state = 0.95*astro + 0.05*ss/(32*2048)
        ns_t = small.tile([1, 1], F32, tag="ns_t")
        v1 = small.tile([1, 1], F32, tag="v1")
        nc.vector.tensor_scalar_mul(v1, ss, (1.0 - ASTRO_DECAY) / (32.0 * TQ))
        v2 = small.tile([1, 1], F32, tag="v2")
        nc.vector.tensor_scalar_mul(v2, scal_sb[0:1, 0:1], ASTRO_DECAY)
        nc.vector.tensor_add(ns_t, v1, v2)
        nc.gpsimd.dma_start(ns_out[:], ns_t)
        # c = tau/4 = max(1 + astro_scale*ns, 0.001)/4
        c_t = small.tile([1, 1], F32, tag="c_t")
        nc.vector.tensor_mul(c_t, ns_t, scal_sb[0:1, 1:2])
        nc.vector.tensor_scalar(c_t, c_t, 1.0, 0.001, ALU.add, ALU.max)
        nc.vector.tensor_scalar_mul(c_t, c_t, 0.25 * TAU_BASE)
        nc.gpsimd.dma_start(c_stage[:], c_t)
        c_sb = small.tile([P, 1], F32, tag="c_sb")
        nc.sync.dma_start(
            c_sb, bass.AP(tensor=c_stage, offset=0, ap=[[0, P], [1, 1]]))
        for j in range(NJ):
            nc.vector.tensor_mul(qn[:, j, :], qn[:, j, :], rq_bc)

        # ==================== S_H/S_I: scores + partial out per query chunk
        for qc in range(NQT):
            scT = scorepool.tile([P, NKT2, 512], BF, tag="scT", name=f"scT{qc}")
            rs_ps = psrow.tile([1, 512], F32, tag="rowsum", name=f"rs_ps{qc}")
            for kt in range(NKT2):
                ps_s = ps.tile([P, 512], F32, tag="mm")
                for j in range(NJ):
                    nc.tensor.matmul(ps_s, kn[:, j, kt * P:(kt + 1) * P],
                                     qn[:, j, qc * 512:(qc + 1) * 512],
                                     start=(j == 0), stop=(j == NJ - 1))
                u0 = tmp.tile([P, 512], BF, tag="u0")
                # u0 = 1 - cos = S*(-rk[key]) + 1
                nc.vector.tensor_scalar(u0, ps_s, nrk_col[:, kt:kt + 1], 1.0,
                                        ALU.mult, ALU.add)
                u1 = tmp.tile([P, 512], BF, tag="u1")
                nc.scalar.activation(u1, u0, AF.Square)
                u2 = tmp.tile([P, 512], BF, tag="u2")
                nc.vector.tensor_scalar_mul(u2, u1, c_sb)
                # scores = relu(1 - c*(1-cos)^2)
                nc.scalar.activation(scT[:, kt, :], u2, AF.Relu,
                                     bias=1.0, scale=-1.0)
                nc.tensor.matmul(rs_ps, ones_k, scT[:, kt, :],
                                 start=(kt == 0), stop=(kt == NKT2 - 1))
            # stage this chunk's partial row sums for the AllReduce
            rs_row = rows.tile([1, 512], F32, tag="rowa", name="rs_row")
            nc.vector.tensor_copy(rs_row, rs_ps)
            nc.gpsimd.dma_start(rs_in[0:1, qc * 512:(qc + 1) * 512], rs_row)
            # partial out = VW.T @ scT for this chunk
            for j2g in range(2):
                pcs = [ps.tile([P, 512], F32, tag="mm", name=f"pcs{i}") for i in range(4)]
                for kt in range(NKT2):
                    for jj in range(4):
                        nc.tensor.matmul(pcs[jj],
                                         VW[:, kt, j2g * 512 + jj * P:j2g * 512 + (jj + 1) * P],
                                         scT[:, kt, :],
                                         start=(kt == 0), stop=(kt == NKT2 - 1))
                for jj in range(4):
                    j = j2g * 4 + jj
                    o_t = outstage.tile([P, 512], BF, tag="outstage")
                    nc.scalar.activation(o_t, pcs[jj], AF.Copy)
                    nc.gpsimd.dma_start(po_in[qc][j * P:(j + 1) * P, :], o_t)
            # pair AllReduce for this chunk's partial out (pipelined)
            nc.gpsimd.collective_compute(
                "AllReduce", ALU.add, replica_groups=CC_GROUPS,
                ins=[po_in[qc][:]], outs=[po_out[qc][:]])
        # row-sum AllReduce (tiny, once)
        nc.gpsimd.collective_compute(
            "AllReduce", ALU.add, replica_groups=CC_GROUPS,
            ins=[rs_in[:]], outs=[rs_out[:]])

        # ==================== S_K: finalize (full query range; host picks half)
        rsf_bc = small.tile([P, TQ], F32, tag="rsf_bc")
        for qc in range(NQT):
            rs_sl = rows.tile([1, 512], F32, tag="rowa", name="rs_sl")
            nc.sync.dma_start(rs_sl, rs_out[0:1, qc * 512:(qc + 1) * 512])
            rinv = rows.tile([1, 512], F32, tag="rowb", name="rinv")
            nc.vector.reciprocal(rinv, rs_sl)
            nc.gpsimd.dma_start(rsf_stage[0:1, qc * 512:(qc + 1) * 512], rinv)
        nc.sync.dma_start(
            rsf_bc, bass.AP(tensor=rsf_stage, offset=0, ap=[[0, P], [1, TQ]]))
        for qc in range(NQT):
            for j in range(NJ):
                po_sl = attst.tile([P, 512], BF, tag="po_sl", name="po_sl")
                nc.sync.dma_start(po_sl, po_out[qc][j * P:(j + 1) * P, :])
                o_t = tmp.tile([P, 512], F32, tag="o_t")
                nc.vector.tensor_mul(o_t, po_sl, rsf_bc[:, qc * 512:(qc + 1) * 512])
                nc.vector.tensor_scalar_add(o_t, o_t, bvo_sb[:, j:j + 1])
                nc.gpsimd.dma_start(
                    outT[j * P:(j + 1) * P, qc * 512:(qc + 1) * 512], o_t)

    return nc


_CACHE = {}


def _get_nc():
    if "nc" not in _CACHE:
        nc = bacc.Bacc(None, target_bir_lowering=False)
        _emit(nc)
        nc.finalize()
        _CACHE["nc"] = nc
    return _CACHE["nc"]


def build_in_maps(inputs):
    """Host-side prep: foldings, packing, per-core sharding."""
    F, G = _build_dft_mats(D)
    role = np.asarray(inputs["role_matrix"], np.float32)
    role = role / np.clip(np.linalg.norm(role, axis=-1, keepdims=True), 1e-12, None)
    PF = _pack_rfft(role)

    f32 = lambda x: np.asarray(x, np.float32)
    bf = lambda x: np.ascontiguousarray(f32(x)).astype(BF16)
    btile = lambda x: np.ascontiguousarray(f32(x).reshape(-1, P).T.copy())

    Wk, Wv, Wo, Wr1 = f32(inputs["Wk"]), f32(inputs["Wv"]), f32(inputs["Wo"]), f32(inputs["Wr1"])
    bk, bv, bo, br1 = f32(inputs["bk"]), f32(inputs["bv"]), f32(inputs["bo"]), f32(inputs["br1"])

    weights = {
        "wq": bf(inputs["Wq"]),
        "wkf": bf(Wk @ F),
        "gmat": bf(G),
        "vwo": bf(Wv @ Wo),
        "wkr1": bf(Wk @ Wr1),
        "wr2": bf(inputs["Wr2"]),
        "pf": bf(PF),
        "bq": btile(inputs["bq"]),
        "bkf": btile(bk @ F),
        "bvo": btile(bv @ Wo + bo),
        "brk": btile(bk @ Wr1 + br1),
    }

    in_maps = []
    for core in range(8):
        b, h = core // 2, core % 2
        m = dict(weights)
        m["qT"] = np.ascontiguousarray(f32(inputs["q_in"][b]).T).astype(BF16)
        m["kT"] = np.ascontiguousarray(
            f32(inputs["k_in"][b])[h * TK2:(h + 1) * TK2].T).astype(BF16)
        m["vT"] = np.ascontiguousarray(
            f32(inputs["v_in"][b])[h * TK2:(h + 1) * TK2].T).astype(BF16)
        m["scal"] = np.array(
            [[np.float32(inputs["astrocyte_state"][b]),
              np.float32(np.asarray(inputs["astro_scale"]).reshape(-1)[0])]],
            np.float32)
        in_maps.append(m)
    return in_maps


def kernel(q_in, k_in, v_in, astrocyte_state, Wq, bq, Wk, bk, Wv, bv, Wo, bo,
           role_matrix, Wr1, br1, Wr2, astro_scale, **_ignored):
    nc = _get_nc()
    inputs = dict(q_in=q_in, k_in=k_in, v_in=v_in, astrocyte_state=astrocyte_state,
                  Wq=Wq, bq=bq, Wk=Wk, bk=bk, Wv=Wv, bv=bv, Wo=Wo, bo=bo,
                  role_matrix=role_matrix, Wr1=Wr1, br1=br1, Wr2=Wr2,
                  astro_scale=astro_scale)
    in_maps = build_in_maps(inputs)
    res = run_bass_kernel_spmd(nc, in_maps, core_ids=list(range(8)))

    output = np.empty((B, TQ, D), np.float32)
    new_state = np.empty((B,), np.float32)
    for b in range(B):
        output[b] = res.results[2 * b]["outT"].T
        new_state[b] = res.results[2 * b]["ns_out"][0, 0]
    return output, new_state


# revision 29
# speedup vs baseline: 1.0752x; 1.0752x over previous
"""Trainium2 Bass kernel for nn_AstroSymbolicEpisodicLayer.

Strategy
--------
8 cores = (batch b in 0..3) x (key-half h in 0..1).  Each core computes the
full 2048-query attention against its 1024-key half; the pair combines
partial (unnormalized) outputs and score row-sums with pair-local
AllReduces pipelined per query chunk, then finalizes the full output (the
host reads one core per batch).  One SPMD program; per-core differences are
pure data.

Math decomposition (validated to ~1e-6 in fp32 vs the jax reference):
 - Circular-convolution binding via a packed real DFT implemented as
   1024x1024 matmuls.  Packed layout: rows 0..511 are Re bins 0..511, row 512
   is Re bin 512, rows 513..1023 are Im bins 1..511.  Pointwise complex
   multiply pairs row f with row 512+f (tile j with tile j+4), plus a 2-row
   fixup for the purely-real bins 0 and 512.
 - rfft(role_vecs) = role_weights @ rfft(role_norm) by linearity; the softmax
   denominator cancels in the K_bound l2-normalization so we use plain exp.
 - Host weight foldings remove two full projection stages: K is never
   materialized (Kf = kT.T @ (Wk F) + bk F and hidden = relu(kT.T @ (Wk Wr1)
   + bk Wr1 + br1)), and the output projection folds into V (VW = vT.T @
   (Wv Wo); the bv term becomes a constant output bias because the attention
   rows are normalized).
 - Per-token normalizations are folded: 1/||K_bound|| becomes a per-partition
   scalar in the scores epilogue (scores computed keys-major), 1/||Q|| scales
   q_n in SBUF, and the attention row-sum is folded into the output epilogue
   (after the pair AllReduce).

Everything is feature-major ("xT" = [feature, token]) so every chained matmul
is out = lhsT.T @ rhs with contraction on the partition dim.
"""

import sys

for _p in ("/opt/trn_rl_repo", "/root/.axon_site/_ro/trn_rl_repo"):
    if _p not in sys.path:
        sys.path.append(_p)

import numpy as np
import ml_dtypes

import concourse.bass as bass
import concourse.bacc as bacc
import concourse.tile as tile
from concourse import mybir
from concourse.bass_utils import run_bass_kernel_spmd

BF16 = ml_dtypes.bfloat16
F32 = mybir.dt.float32
BF = mybir.dt.bfloat16

B, TQ, TK, D, R, H = 4, 2048, 2048, 1024, 64, 512
TAU_BASE, ASTRO_DECAY = 1.0, 0.95
P = 128
NJ = D // P            # 8 feature chunks
TK2 = TK // 2          # keys per core (1024)
NT2 = TK2 // 512       # 2 key token chunks
NKT2 = TK2 // P        # 8 key tiles of 128
NQT = TQ // 512        # 4 query chunks of 512
TQH = TQ // 2
AF = mybir.ActivationFunctionType
ALU = mybir.AluOpType
CC_GROUPS = [[0, 1], [2, 3], [4, 5], [6, 7]]


# ---------------------------------------------------------------- host consts
def _build_dft_mats(n=1024):
    j = np.arange(n)[:, None].astype(np.float64)
    f = np.arange(n // 2)[None, :].astype(np.float64)
    F = np.zeros((n, n), np.float64)
    F[:, :512] = np.cos(2 * np.pi * j * f / n)
    F[:, 512] = np.cos(np.pi * j[:, 0])
    fi = np.arange(1, 512)[None, :].astype(np.float64)
    F[:, 513:] = -np.sin(2 * np.pi * j * fi / n)

    G = np.zeros((n, n), np.float64)
    d = np.arange(n)[None, :].astype(np.float64)
    G[0, :] = 1.0 / n
    ff = np.arange(1, 512)[:, None].astype(np.float64)
    G[1:512, :] = 2.0 * np.cos(2 * np.pi * ff * d / n) / n
    G[512, :] = ((-1.0) ** d[0]) / n
    G[513:, :] = -2.0 * np.sin(2 * np.pi * ff * d / n) / n
    return F.astype(np.float32), G.astype(np.float32)


def _pack_rfft(x):
    X = np.fft.rfft(np.asarray(x, np.float64), axis=-1)
    out = np.empty(x.shape, np.float32)
    out[..., :512] = X.real[..., :512]
    out[..., 512] = X.real[..., 512]
    out[..., 513:] = X.imag[..., 1:512]
    return out


# ---------------------------------------------------------------- bass kernel
def _emit(nc: bass.Bass):
    # ---- I/O declarations (names = in_map keys) ----
    din = {}
    for nm, shp, dt in [
        ("qT", [D, TQ], BF), ("kT", [D, TK2], BF), ("vT", [D, TK2], BF),
        ("wq", [D, D], BF), ("wkf", [D, D], BF), ("gmat", [D, D], BF),
        ("vwo", [D, D], BF), ("wkr1", [D, H], BF), ("wr2", [H, R], BF),
        ("pf", [R, D], BF),
        ("bq", [P, NJ], F32), ("bkf", [P, NJ], F32), ("bvo", [P, NJ], F32),
        ("brk", [P, H // P], F32), ("scal", [1, 2], F32),
    ]:
        din[nm] = nc.declare_dram_parameter(nm, shp, dt, isOutput=False)
    outT = nc.declare_dram_parameter("outT", [D, TQ], F32, isOutput=True)
    ns_out = nc.declare_dram_parameter("ns_out", [1, 1], F32, isOutput=True)

    # internal DRAM: broadcast staging + collective buffers (per query chunk)
    rk_stage = nc.dram_tensor("rk_stage", [1, TK2], F32)
    rq_stage = nc.dram_tensor("rq_stage", [1, TQ], BF)
    rsf_stage = nc.dram_tensor("rsf_stage", [1, TQ], F32)
    c_stage = nc.dram_tensor("c_stage", [1, 1], F32)
    po_in = [nc.dram_tensor(f"po_in{qc}", [D, 512], BF) for qc in range(NQT)]
    po_out = [nc.dram_tensor(f"po_out{qc}", [D, 512], BF) for qc in range(NQT)]
    rs_in = nc.dram_tensor("rs_in", [1, TQ], F32)
    rs_out = nc.dram_tensor("rs_out", [1, TQ], F32)

    c3 = lambda t: t.rearrange("(po pi) x -> pi po x", pi=P)

    with tile.TileContext(nc) as tc, \
         tc.tile_pool(name="w", bufs=2) as wpool, \
         tc.tile_pool(name="wsmall", bufs=1) as wsmall, \
         tc.tile_pool(name="kinvw", bufs=1) as kinvw, \
         tc.tile_pool(name="qstream", bufs=2) as qstream, \
         tc.tile_pool(name="outstage", bufs=2) as outstage, \
         tc.tile_pool(name="persist", bufs=1) as persist, \
         tc.tile_pool(name="kfc", bufs=2) as kfcpool, \
         tc.tile_pool(name="scores", bufs=2) as scorepool, \
         tc.tile_pool(name="attst", bufs=3) as attst, \
         tc.tile_pool(name="tmp", bufs=2) as tmp, \
         tc.tile_pool(name="cmt", bufs=2) as cmt, \
         tc.tile_pool(name="small", bufs=1) as small, \
         tc.tile_pool(name="rows", bufs=1) as rows, \
         tc.tile_pool(name="ps", bufs=4, space="PSUM") as ps, \
         tc.tile_pool(name="psrow", bufs=3, space="PSUM") as psrow:

        # ---- constants ----
        ones_k = small.tile([P, 1], BF, tag="ones_k")      # lhsT for partition sums
        nc.vector.memset(ones_k, 1.0)
        bq_sb = small.tile([P, NJ], F32, tag="bq")
        nc.sync.dma_start(bq_sb, din["bq"][:])
        bkf_sb = small.tile([P, NJ], F32, tag="bkf")
        nc.sync.dma_start(bkf_sb, din["bkf"][:])
        bvo_sb = small.tile([P, NJ], F32, tag="bvo")
        nc.sync.dma_start(bvo_sb, din["bvo"][:])
        brk_sb = small.tile([P, H // P], F32, tag="brk")
        nc.sync.dma_start(brk_sb, din["brk"][:])
        scal_sb = small.tile([1, 2], F32, tag="scal")
        nc.sync.dma_start(scal_sb, din["scal"][:])

        # pre-touch const tiles (ACT has a small hardware wait table)
        pre_s = small.tile([P, 1], F32, tag="pre_s")
        nc.scalar.activation(pre_s, bq_sb[:, 0:1], AF.Identity, bias=bkf_sb[:, 0:1])
        nc.scalar.activation(pre_s, pre_s, AF.Relu, bias=brk_sb[:, 0:1])
        pre_v = small.tile([P, 1], F32, tag="pre_v")
        nc.vector.tensor_scalar_add(pre_v, pre_s, bvo_sb[:, 0:1])
        pre_v2 = small.tile([1, 1], F32, tag="pre_v2")
        nc.vector.tensor_scalar_add(pre_v2, scal_sb[0:1, 0:1], 0.0)

        def load_w(name, shape3):
            t = wpool.tile(shape3, BF, tag="w", name=name)
            nc.gpsimd.dma_start(t, c3(din[name][:]))
            return t

        # kT half resident (used by both the hidden pass and the Kf pass)
        kin = kinvw.tile([P, NJ, TK2], BF, tag="kinvw", name="kin")
        nc.sync.dma_start(kin[:, :, :512], c3(din["kT"][:])[:, :, :512])
        nc.sync.dma_start(kin[:, :, 512:], c3(din["kT"][:])[:, :, 512:])

        # ==================== S_A: hidden = relu(WkR1.T @ kT + brk)
        wkr1 = wsmall.tile([P, NJ, H], BF, tag="wkr1")
        nc.sync.dma_start(wkr1, c3(din["wkr1"][:]))
        hidden = scorepool.tile([P, H // P, TK2], BF, tag="scT", name="hidden")
        for t in range(NT2):
            for jh in range(H // P):
                pst = ps.tile([P, 512], F32, tag="mm")
                for i in range(NJ):
                    nc.tensor.matmul(pst, wkr1[:, i, jh * P:(jh + 1) * P],
                                     kin[:, i, t * 512:(t + 1) * 512],
                                     start=(i == 0), stop=(i == NJ - 1))
                nc.scalar.activation(hidden[:, jh, t * 512:(t + 1) * 512], pst,
                                     AF.Relu, bias=brk_sb[:, jh:jh + 1])

        # ==================== S_C: role logits -> exp -> Rf (SBUF resident)
        wr2 = wsmall.tile([P, H // P, R], BF, tag="wr2")
        nc.gpsimd.dma_start(wr2, c3(din["wr2"][:]))
        rw = persist.tile([R, TK2], BF, tag="rw")
        for t in range(NT2):
            ps64 = ps.tile([R, 512], F32, tag="mm")
            for i2 in range(H // P):
                nc.tensor.matmul(ps64, wr2[:, i2, :], hidden[:, i2, t * 512:(t + 1) * 512],
                                 start=(i2 == 0), stop=(i2 == H // P - 1))
            nc.scalar.activation(rw[:, t * 512:(t + 1) * 512], ps64, AF.Exp)
        pf = wsmall.tile([R, D], BF, tag="pf")
        nc.gpsimd.dma_start(pf, din["pf"][:])
        Rf = persist.tile([P, NJ, TK2], BF, tag="big1", name="Rf")
        for t in range(NT2):
            for j in range(NJ):
                pst = ps.tile([P, 512], F32, tag="mm")
                nc.tensor.matmul(pst, pf[:, j * P:(j + 1) * P],
                                 rw[:, t * 512:(t + 1) * 512], start=True, stop=True)
                nc.scalar.activation(Rf[:, j, t * 512:(t + 1) * 512], pst, AF.Copy)

        # ==================== S_B: fused Kf -> cmul -> KB per key chunk
        wkf = load_w("wkf", [P, NJ, D])
        gmat = load_w("gmat", [P, NJ, D])
        kn = persist.tile([P, NJ, TK2], BF, tag="kn")
        kfc = [None] * NT2

        def emit_kf(t):
            kfc[t] = kfcpool.tile([P, NJ, 512], BF, tag="kfc", name=f"kfc{t}")
            for j in range(NJ):
                pst = ps.tile([P, 512], F32, tag="mm")
                for i in range(NJ):
                    nc.tensor.matmul(pst, wkf[:, i, j * P:(j + 1) * P],
                                     kin[:, i, t * 512:(t + 1) * 512],
                                     start=(i == 0), stop=(i == NJ - 1))
                nc.scalar.activation(kfc[t][:, j, :], pst, AF.Identity,
                                     bias=bkf_sb[:, j:j + 1])
            # cmul in place: kf chunk becomes Z chunk (DVE)
            kf = kfc[t]
            rfs = Rf[:, :, t * 512:(t + 1) * 512]
            zf0 = cmt.tile([1, 512], BF, tag="zf0")
            zf4 = cmt.tile([1, 512], BF, tag="zf4")
            nc.vector.tensor_mul(zf0, kf[0:1, 0, :], rfs[0:1, 0, :])
            nc.vector.tensor_mul(zf4, kf[0:1, 4, :], rfs[0:1, 4, :])
            for j in range(4):
                t2 = cmt.tile([P, 512], BF, tag="cm_t2")
                t3 = cmt.tile([P, 512], BF, tag="cm_t3")
                nc.vector.tensor_mul(t2, kf[:, j + 4, :], rfs[:, j + 4, :])
                nc.vector.tensor_mul(t3, kf[:, j + 4, :], rfs[:, j, :])
                nc.vector.tensor_mul(kf[:, j + 4, :], kf[:, j, :], rfs[:, j + 4, :])
                nc.vector.tensor_tensor(kf[:, j + 4, :], kf[:, j + 4, :], t3, ALU.add)
                nc.vector.tensor_mul(kf[:, j, :], kf[:, j, :], rfs[:, j, :])
                nc.vector.tensor_tensor(kf[:, j, :], kf[:, j, :], t2, ALU.subtract)
            # rows 0 (re_0) and 512 (re_512) are purely real
            nc.vector.tensor_copy(kf[0:1, 0, :], zf0)
            nc.vector.tensor_copy(kf[0:1, 4, :], zf4)

        def emit_kb(t):
            psn = psrow.tile([1, 512], F32, tag="rowsum", name=f"psn_kb{t}")
            for j in range(NJ):
                pst = ps.tile([P, 512], F32, tag="mm")
                for i in range(NJ):
                    nc.tensor.matmul(pst, gmat[:, i, j * P:(j + 1) * P], kfc[t][:, i, :],
                                     start=(i == 0), stop=(i == NJ - 1))
                kb = kn[:, j, t * 512:(t + 1) * 512]
                nc.scalar.activation(kb, pst, AF.Copy)
                sq = tmp.tile([P, 512], BF, tag="sq")
                nc.vector.tensor_mul(sq, kb, kb)
                nc.tensor.matmul(psn, ones_k, sq, start=(j == 0), stop=(j == NJ - 1))
            # rk chunk = 1/sqrt(norm2) -> stage to DRAM for the column reload
            srow = rows.tile([1, 512], F32, tag="rowa", name="srow")
            nc.scalar.activation(srow, psn, AF.Sqrt)
            rrow = rows.tile([1, 512], F32, tag="rowb", name="rrow")
            nc.vector.reciprocal(rrow, srow)
            nc.gpsimd.dma_start(rk_stage[0:1, t * 512:(t + 1) * 512], rrow)

        emit_kf(0)
        emit_kf(1)
        emit_kb(0)
        emit_kb(1)
        # rk as [128,8] per-key-tile partition scalars, negated for the
        # (S*(-rk) + 1) epilogue
        rk_col = small.tile([P, NKT2], F32, tag="rk_col")
        nc.sync.dma_start(
            rk_col, rk_stage.rearrange("o (kt p) -> (o p) kt", p=P))
        nrk_col = small.tile([P, NKT2], F32, tag="nrk_col")
        nc.vector.tensor_scalar_mul(nrk_col, rk_col, -1.0)

        # ==================== S_E: VW = vT.T @ (WvWo), SBUF resident
        vwo = load_w("vwo", [P, NJ, D])
        VW = kinvw.tile([P, NKT2, D], BF, tag="kinvw", name="VW")
        for to in range(NKT2):
            vin = attst.tile([P, NJ, P], BF, tag="vin", name="vin")
            nc.sync.dma_start(vin, c3(din["vT"][:])[:, :, to * P:(to + 1) * P])
            for n in range(2):
                pst = ps.tile([P, 512], F32, tag="mm")
                for i in range(NJ):
                    nc.tensor.matmul(pst, vin[:, i, :], vwo[:, i, n * 512:(n + 1) * 512],
                                     start=(i == 0), stop=(i == NJ - 1))
                nc.scalar.activation(VW[:, to, n * 512:(n + 1) * 512], pst, AF.Copy)

        # ==================== S_F: Q proj (full TQ), tau, q_n
        wq = load_w("wq", [P, NJ, D])
        qn = persist.tile([P, NJ, TQ], BF, tag="big1", name="qn")
        rq_bc = small.tile([P, TQ], BF, tag="rq_bc")
        ssc = small.tile([1, NQT], F32, tag="ssc")
        for t in range(NQT):
            qin = qstream.tile([P, NJ, 512], BF, tag="qstream", name="qin")
            nc.sync.dma_start(qin, c3(din["qT"][:])[:, :, t * 512:(t + 1) * 512])
            psn = psrow.tile([1, 512], F32, tag="rowsum")
            for j in range(NJ):
                pst = ps.tile([P, 512], F32, tag="mm")
                for i in range(NJ):
                    nc.tensor.matmul(pst, wq[:, i, j * P:(j + 1) * P], qin[:, i, :],
                                     start=(i == 0), stop=(i == NJ - 1))
                qv = qn[:, j, t * 512:(t + 1) * 512]
                nc.scalar.activation(qv, pst, AF.Identity, bias=bq_sb[:, j:j + 1])
                sq = tmp.tile([P, 512], BF, tag="sq")
                nc.vector.tensor_mul(sq, qv, qv)
                nc.tensor.matmul(psn, ones_k, sq, start=(j == 0), stop=(j == NJ - 1))
            # sqrt of norms; accumulate sum(sqrt) for the surprise mean
            srow = rows.tile([1, 512], F32, tag="rowa", name="srow")
            nc.scalar.activation(srow, psn, AF.Sqrt, accum_out=ssc[0:1, t:t + 1])
            rrow = rows.tile([1, 512], F32, tag="rowb", name="rrow")
            nc.vector.reciprocal(rrow, srow)
            rrow16 = rows.tile([1, 512], BF, tag="rrow16")
            nc.vector.tensor_copy(rrow16, rrow)
            nc.gpsimd.dma_start(rq_stage[0:1, t * 512:(t + 1) * 512], rrow16)
        nc.sync.dma_start(
            rq_bc, bass.AP(tensor=rq_stage, offset=0, ap=[[0, P], [1, TQ]]))
        ss = small.tile([1, 1], F32, tag="ss")
        nc.vector.reduce_sum(ss, ssc, axis=mybir.AxisListType.X)
        # new_state = 0.95*astro + 0.05*ss/(32*2048)
        ns_t = small.tile([1, 1], F32, tag="ns_t")
        v1 = small.tile([1, 1], F32, tag="v1")
        nc.vector.tensor_scalar_mul(v1, ss, (1.0 - ASTRO_DECAY) / (32.0 * TQ))
        v2 = small.tile([1, 1], F32, tag="v2")
        nc.vector.tensor_scalar_mul(v2, scal_sb[0:1, 0:1], ASTRO_DECAY)
        nc.vector.tensor_add(ns_t, v1, v2)
        nc.gpsimd.dma_start(ns_out[:], ns_t)
        # c = tau/4 = max(1 + astro_scale*ns, 0.001)/4
        c_t = small.tile([1, 1], F32, tag="c_t")
        nc.vector.tensor_mul(c_t, ns_t, scal_sb[0:1, 1:2])
        nc.vector.tensor_scalar(c_t, c_t, 1.0, 0.001, ALU.add, ALU.max)
        nc.vector.tensor_scalar_mul(c_t, c_t, 0.25 * TAU_BASE)
        nc.gpsimd.dma_start(c_stage[:], c_t)
        c_sb = small.tile([P, 1], F32, tag="c_sb")
        nc.sync.dma_start(
            c_sb, bass.AP(tensor=c_stage, offset=0, ap=[[0, P], [1, 1]]))
        for j in range(NJ):
            nc.vector.tensor_mul(qn[:, j, :], qn[:, j, :], rq_bc)

        # ==================== S_H/S_I: scores + partial out per query chunk
        for qc in range(NQT):
            scT = scorepool.tile([P, NKT2, 512], BF, tag="scT", name=f"scT{qc}")
            rs_ps = psrow.tile([1, 512], F32, tag="rowsum", name=f"rs_ps{qc}")
            for kt in range(NKT2):
                ps_s = ps.tile([P, 512], F32, tag="mm")
                for j in range(NJ):
                    nc.tensor.matmul(ps_s, kn[:, j, kt * P:(kt + 1) * P],
                                     qn[:, j, qc * 512:(qc + 1) * 512],
                                     start=(j == 0), stop=(j == NJ - 1))
                u0 = tmp.tile([P, 512], BF, tag="u0")
                # u0 = 1 - cos = S*(-rk[key]) + 1
                nc.vector.tensor_scalar(u0, ps_s, nrk_col[:, kt:kt + 1], 1.0,
                                        ALU.mult, ALU.add)
                u1 = tmp.tile([P, 512], BF, tag="u1")
                nc.scalar.activation(u1, u0, AF.Square)
                u2 = tmp.tile([P, 512], BF, tag="u2")
                nc.vector.tensor_scalar_mul(u2, u1, c_sb)
                # scores = relu(1 - c*(1-cos)^2)
                nc.scalar.activation(scT[:, kt, :], u2, AF.Relu,
                                     bias=1.0, scale=-1.0)
                nc.tensor.matmul(rs_ps, ones_k, scT[:, kt, :],
                                 start=(kt == 0), stop=(kt == NKT2 - 1))
            # stage this chunk's partial row sums for the AllReduce
            rs_row = rows.tile([1, 512], F32, tag="rowa", name="rs_row")
            nc.vector.tensor_copy(rs_row, rs_ps)
            nc.gpsimd.dma_start(rs_in[0:1, qc * 512:(qc + 1) * 512], rs_row)
            if qc == NQT - 1:
                # all partial row sums staged: combine early so the
                # epilogue isn't gated on a tail collective
                nc.gpsimd.collective_compute(
                    "AllReduce", ALU.add, replica_groups=CC_GROUPS,
                    ins=[rs_in[:]], outs=[rs_out[:]])
            # partial out = VW.T @ scT for this chunk
            for j2g in range(2):
                pcs = [ps.tile([P, 512], F32, tag="mm", name=f"pcs{i}") for i in range(4)]
                for kt in range(NKT2):
                    for jj in range(4):
                        nc.tensor.matmul(pcs[jj],
                                         VW[:, kt, j2g * 512 + jj * P:j2g * 512 + (jj + 1) * P],
                                         scT[:, kt, :],
                                         start=(kt == 0), stop=(kt == NKT2 - 1))
                for jj in range(4):
                    j = j2g * 4 + jj
                    o_t = outstage.tile([P, 512], BF, tag="outstage")
                    nc.scalar.activation(o_t, pcs[jj], AF.Copy)
                    nc.gpsimd.dma_start(po_in[qc][j * P:(j + 1) * P, :], o_t)
            # pair AllReduce for this chunk's partial out (pipelined)
            nc.gpsimd.collective_compute(
                "AllReduce", ALU.add, replica_groups=CC_GROUPS,
                ins=[po_in[qc][:]], outs=[po_out[qc][:]])
        # ==================== S_K: finalize (full query range; host picks half)
        for qc in range(NQT):
            rs_sl = rows.tile([1, 512], F32, tag="rowa", name="rs_sl")
            nc.sync.dma_start(rs_sl, rs_out[0:1, qc * 512:(qc + 1) * 512])
            rinv = rows.tile([1, 512], F32, tag="rowb", name="rinv")
            nc.vector.reciprocal(rinv, rs_sl)
            nc.gpsimd.dma_start(rsf_stage[0:1, qc * 512:(qc + 1) * 512], rinv)
            rsf_bc = small.tile([P, 512], F32, tag="rsf_bc", name=f"rsf_bc{qc}")
            nc.sync.dma_start(
                rsf_bc, bass.AP(tensor=rsf_stage, offset=qc * 512,
                                ap=[[0, P], [1, 512]]))
            for j in range(NJ):
                po_sl = attst.tile([P, 512], BF, tag="po_sl", name="po_sl")
                nc.sync.dma_start(po_sl, po_out[qc][j * P:(j + 1) * P, :])
                o_t = tmp.tile([P, 512], F32, tag="o_t")
                nc.vector.tensor_mul(o_t, po_sl, rsf_bc)
                nc.vector.tensor_scalar_add(o_t, o_t, bvo_sb[:, j:j + 1])
                nc.gpsimd.dma_start(
                    outT[j * P:(j + 1) * P, qc * 512:(qc + 1) * 512], o_t)

    return nc


_CACHE = {}


def _get_nc():
    if "nc" not in _CACHE:
        nc = bacc.Bacc(None, target_bir_lowering=False)
        _emit(nc)
        nc.finalize()
        _CACHE["nc"] = nc
    return _CACHE["nc"]


def build_in_maps(inputs):
    """Host-side prep: foldings, packing, per-core sharding."""
    F, G = _build_dft_mats(D)
    role = np.asarray(inputs["role_matrix"], np.float32)
    role = role / np.clip(np.linalg.norm(role, axis=-1, keepdims=True), 1e-12, None)
    PF = _pack_rfft(role)

    f32 = lambda x: np.asarray(x, np.float32)
    bf = lambda x: np.ascontiguousarray(f32(x)).astype(BF16)
    btile = lambda x: np.ascontiguousarray(f32(x).reshape(-1, P).T.copy())

    Wk, Wv, Wo, Wr1 = f32(inputs["Wk"]), f32(inputs["Wv"]), f32(inputs["Wo"]), f32(inputs["Wr1"])
    bk, bv, bo, br1 = f32(inputs["bk"]), f32(inputs["bv"]), f32(inputs["bo"]), f32(inputs["br1"])

    weights = {
        "wq": bf(inputs["Wq"]),
        "wkf": bf(Wk @ F),
        "gmat": bf(G),
        "vwo": bf(Wv @ Wo),
        "wkr1": bf(Wk @ Wr1),
        "wr2": bf(inputs["Wr2"]),
        "pf": bf(PF),
        "bq": btile(inputs["bq"]),
        "bkf": btile(bk @ F),
        "bvo": btile(bv @ Wo + bo),
        "brk": btile(bk @ Wr1 + br1),
    }

    in_maps = []
    for core in range(8):
        b, h = core // 2, core % 2
        m = dict(weights)
        m["qT"] = np.ascontiguousarray(f32(inputs["q_in"][b]).T).astype(BF16)
        m["kT"] = np.ascontiguousarray(
            f32(inputs["k_in"][b])[h * TK2:(h + 1) * TK2].T).astype(BF16)
        m["vT"] = np.ascontiguousarray(
            f32(inputs["v_in"][b])[h * TK2:(h + 1) * TK2].T).astype(BF16)
        m["scal"] = np.array(
            [[np.float32(inputs["astrocyte_state"][b]),
              np.float32(np.asarray(inputs["astro_scale"]).reshape(-1)[0])]],
            np.float32)
        in_maps.append(m)
    return in_maps


def kernel(q_in, k_in, v_in, astrocyte_state, Wq, bq, Wk, bk, Wv, bv, Wo, bo,
           role_matrix, Wr1, br1, Wr2, astro_scale, **_ignored):
    nc = _get_nc()
    inputs = dict(q_in=q_in, k_in=k_in, v_in=v_in, astrocyte_state=astrocyte_state,
                  Wq=Wq, bq=bq, Wk=Wk, bk=bk, Wv=Wv, bv=bv, Wo=Wo, bo=bo,
                  role_matrix=role_matrix, Wr1=Wr1, br1=br1, Wr2=Wr2,
                  astro_scale=astro_scale)
    in_maps = build_in_maps(inputs)
    res = run_bass_kernel_spmd(nc, in_maps, core_ids=list(range(8)))

    output = np.empty((B, TQ, D), np.float32)
    new_state = np.empty((B,), np.float32)
    for b in range(B):
        output[b] = res.results[2 * b]["outT"].T
        new_state[b] = res.results[2 * b]["ns_out"][0, 0]
    return output, new_state


# revision 32
# speedup vs baseline: 1.1096x; 1.0320x over previous
"""Trainium2 Bass kernel for nn_AstroSymbolicEpisodicLayer.

Strategy
--------
8 cores = (batch b in 0..3) x (key-half h in 0..1).  Each core computes the
full 2048-query attention against its 1024-key half; the pair combines
partial (unnormalized) outputs and score row-sums with pair-local
AllReduces pipelined per query chunk, then finalizes the full output (the
host reads one core per batch).  One SPMD program; per-core differences are
pure data.

Math decomposition (validated to ~1e-6 in fp32 vs the jax reference):
 - Circular-convolution binding via a packed real DFT implemented as
   1024x1024 matmuls.  Packed layout: rows 0..511 are Re bins 0..511, row 512
   is Re bin 512, rows 513..1023 are Im bins 1..511.  Pointwise complex
   multiply pairs row f with row 512+f (tile j with tile j+4), plus a 2-row
   fixup for the purely-real bins 0 and 512.
 - rfft(role_vecs) = role_weights @ rfft(role_norm) by linearity; the softmax
   denominator cancels in the K_bound l2-normalization so we use plain exp.
 - Host weight foldings remove two full projection stages: K is never
   materialized (Kf = kT.T @ (Wk F) + bk F and hidden = relu(kT.T @ (Wk Wr1)
   + bk Wr1 + br1)), and the output projection folds into V (VW = vT.T @
   (Wv Wo); the bv term becomes a constant output bias because the attention
   rows are normalized).
 - Per-token normalizations are folded: 1/||K_bound|| becomes a per-partition
   scalar in the scores epilogue (scores computed keys-major), 1/||Q|| scales
   q_n in SBUF, and the attention row-sum is folded into the output epilogue
   (after the pair AllReduce).

Everything is feature-major ("xT" = [feature, token]) so every chained matmul
is out = lhsT.T @ rhs with contraction on the partition dim.
"""

import sys

for _p in ("/opt/trn_rl_repo", "/root/.axon_site/_ro/trn_rl_repo"):
    if _p not in sys.path:
        sys.path.append(_p)

import numpy as np
import ml_dtypes

import concourse.bass as bass
import concourse.bacc as bacc
import concourse.tile as tile
from concourse import mybir
from concourse.bass_utils import run_bass_kernel_spmd

BF16 = ml_dtypes.bfloat16
F32 = mybir.dt.float32
BF = mybir.dt.bfloat16

B, TQ, TK, D, R, H = 4, 2048, 2048, 1024, 64, 512
TAU_BASE, ASTRO_DECAY = 1.0, 0.95
P = 128
NJ = D // P            # 8 feature chunks
TK2 = TK // 2          # keys per core (1024)
NT2 = TK2 // 512       # 2 key token chunks
NKT2 = TK2 // P        # 8 key tiles of 128
NQT = TQ // 512        # 4 query chunks of 512
TQH = TQ // 2
AF = mybir.ActivationFunctionType
ALU = mybir.AluOpType
CC_GROUPS = [[0, 1], [2, 3], [4, 5], [6, 7]]


# ---------------------------------------------------------------- host consts
def _build_dft_mats(n=1024):
    j = np.arange(n)[:, None].astype(np.float64)
    f = np.arange(n // 2)[None, :].astype(np.float64)
    F = np.zeros((n, n), np.float64)
    F[:, :512] = np.cos(2 * np.pi * j * f / n)
    F[:, 512] = np.cos(np.pi * j[:, 0])
    fi = np.arange(1, 512)[None, :].astype(np.float64)
    F[:, 513:] = -np.sin(2 * np.pi * j * fi / n)

    G = np.zeros((n, n), np.float64)
    d = np.arange(n)[None, :].astype(np.float64)
    G[0, :] = 1.0 / n
    ff = np.arange(1, 512)[:, None].astype(np.float64)
    G[1:512, :] = 2.0 * np.cos(2 * np.pi * ff * d / n) / n
    G[512, :] = ((-1.0) ** d[0]) / n
    G[513:, :] = -2.0 * np.sin(2 * np.pi * ff * d / n) / n
    return F.astype(np.float32), G.astype(np.float32)


def _pack_rfft(x):
    X = np.fft.rfft(np.asarray(x, np.float64), axis=-1)
    out = np.empty(x.shape, np.float32)
    out[..., :512] = X.real[..., :512]
    out[..., 512] = X.real[..., 512]
    out[..., 513:] = X.imag[..., 1:512]
    return out


# ---------------------------------------------------------------- bass kernel
def _emit(nc: bass.Bass):
    # ---- I/O declarations (names = in_map keys) ----
    din = {}
    for nm, shp, dt in [
        ("qT", [D, TQ], BF), ("kT", [D, TK2], BF), ("vT", [D, TK2], BF),
        ("wq", [D, D], BF), ("wkf", [D, D], BF), ("gmat", [D, D], BF),
        ("vwo", [D, D], BF), ("wkr1", [D, H], BF), ("wr2", [H, R], BF),
        ("pf", [R, D], BF),
        ("bq", [P, NJ], F32), ("bkf", [P, NJ], F32), ("bvo", [P, NJ], F32),
        ("brk", [P, H // P], F32), ("scal", [1, 2], F32),
    ]:
        din[nm] = nc.declare_dram_parameter(nm, shp, dt, isOutput=False)
    outT = nc.declare_dram_parameter("outT", [D, TQ], F32, isOutput=True)
    ns_out = nc.declare_dram_parameter("ns_out", [1, 1], F32, isOutput=True)

    # internal DRAM: broadcast staging + collective buffers (per query chunk)
    rk_stage = nc.dram_tensor("rk_stage", [1, TK2], F32)
    rq_stage = nc.dram_tensor("rq_stage", [1, TQ], BF)
    rsf_stage = nc.dram_tensor("rsf_stage", [1, TQ], F32)
    c_stage = nc.dram_tensor("c_stage", [1, 1], F32)
    po_in = [nc.dram_tensor(f"po_in{qc}", [D, 512], BF) for qc in range(NQT)]
    po_out = [nc.dram_tensor(f"po_out{qc}", [D, 512], BF) for qc in range(NQT)]
    rs_in = [nc.dram_tensor(f"rs_in{qc}", [1, 512], F32) for qc in range(NQT)]
    rs_out = [nc.dram_tensor(f"rs_out{qc}", [1, 512], F32) for qc in range(NQT)]

    c3 = lambda t: t.rearrange("(po pi) x -> pi po x", pi=P)

    with tile.TileContext(nc) as tc, \
         tc.tile_pool(name="w", bufs=2) as wpool, \
         tc.tile_pool(name="wsmall", bufs=1) as wsmall, \
         tc.tile_pool(name="kinvw", bufs=1) as kinvw, \
         tc.tile_pool(name="qstream", bufs=2) as qstream, \
         tc.tile_pool(name="outstage", bufs=2) as outstage, \
         tc.tile_pool(name="persist", bufs=1) as persist, \
         tc.tile_pool(name="kfc", bufs=2) as kfcpool, \
         tc.tile_pool(name="scores", bufs=2) as scorepool, \
         tc.tile_pool(name="attst", bufs=3) as attst, \
         tc.tile_pool(name="tmp", bufs=2) as tmp, \
         tc.tile_pool(name="cmt", bufs=2) as cmt, \
         tc.tile_pool(name="small", bufs=1) as small, \
         tc.tile_pool(name="rows", bufs=1) as rows, \
         tc.tile_pool(name="rsf", bufs=2) as rsfpool, \
         tc.tile_pool(name="ps", bufs=4, space="PSUM") as ps, \
         tc.tile_pool(name="psrow", bufs=3, space="PSUM") as psrow:

        # ---- constants ----
        ones_k = small.tile([P, 1], BF, tag="ones_k")      # lhsT for partition sums
        nc.vector.memset(ones_k, 1.0)
        bq_sb = small.tile([P, NJ], F32, tag="bq")
        nc.sync.dma_start(bq_sb, din["bq"][:])
        bkf_sb = small.tile([P, NJ], F32, tag="bkf")
        nc.sync.dma_start(bkf_sb, din["bkf"][:])
        bvo_sb = small.tile([P, NJ], F32, tag="bvo")
        nc.sync.dma_start(bvo_sb, din["bvo"][:])
        brk_sb = small.tile([P, H // P], F32, tag="brk")
        nc.sync.dma_start(brk_sb, din["brk"][:])
        scal_sb = small.tile([1, 2], F32, tag="scal")
        nc.sync.dma_start(scal_sb, din["scal"][:])

        # pre-touch const tiles (ACT has a small hardware wait table)
        pre_s = small.tile([P, 1], F32, tag="pre_s")
        nc.scalar.activation(pre_s, bq_sb[:, 0:1], AF.Identity, bias=bkf_sb[:, 0:1])
        nc.scalar.activation(pre_s, pre_s, AF.Relu, bias=brk_sb[:, 0:1])
        pre_v = small.tile([P, 1], F32, tag="pre_v")
        nc.vector.tensor_scalar_add(pre_v, pre_s, bvo_sb[:, 0:1])
        pre_v2 = small.tile([1, 1], F32, tag="pre_v2")
        nc.vector.tensor_scalar_add(pre_v2, scal_sb[0:1, 0:1], 0.0)

        def load_w(name, shape3):
            t = wpool.tile(shape3, BF, tag="w", name=name)
            nc.gpsimd.dma_start(t, c3(din[name][:]))
            return t

        # ==================== S_A: hidden = relu(WkR1.T @ kT + brk)
        wkr1 = wsmall.tile([P, NJ, H], BF, tag="wkr1")
        nc.sync.dma_start(wkr1, c3(din["wkr1"][:]))
        # kT half resident (used by both the hidden pass and the Kf pass)
        kin = kinvw.tile([P, NJ, TK2], BF, tag="kinvw", name="kin")
        nc.sync.dma_start(kin[:, :, :512], c3(din["kT"][:])[:, :, :512])
        nc.sync.dma_start(kin[:, :, 512:], c3(din["kT"][:])[:, :, 512:])
        hidden = scorepool.tile([P, H // P, TK2], BF, tag="scT", name="hidden")
        for t in range(NT2):
            for jh in range(H // P):
                pst = ps.tile([P, 512], F32, tag="mm")
                for i in range(NJ):
                    nc.tensor.matmul(pst, wkr1[:, i, jh * P:(jh + 1) * P],
                                     kin[:, i, t * 512:(t + 1) * 512],
                                     start=(i == 0), stop=(i == NJ - 1))
                nc.scalar.activation(hidden[:, jh, t * 512:(t + 1) * 512], pst,
                                     AF.Relu, bias=brk_sb[:, jh:jh + 1])

        # ==================== S_C: role logits -> exp -> Rf (SBUF resident)
        wr2 = wsmall.tile([P, H // P, R], BF, tag="wr2")
        nc.gpsimd.dma_start(wr2, c3(din["wr2"][:]))
        rw = persist.tile([R, TK2], BF, tag="rw")
        for t in range(NT2):
            ps64 = ps.tile([R, 512], F32, tag="mm")
            for i2 in range(H // P):
                nc.tensor.matmul(ps64, wr2[:, i2, :], hidden[:, i2, t * 512:(t + 1) * 512],
                                 start=(i2 == 0), stop=(i2 == H // P - 1))
            nc.scalar.activation(rw[:, t * 512:(t + 1) * 512], ps64, AF.Exp)
        pf = wsmall.tile([R, D], BF, tag="pf")
        nc.gpsimd.dma_start(pf, din["pf"][:])
        Rf = persist.tile([P, NJ, TK2], BF, tag="big1", name="Rf")
        for t in range(NT2):
            for j in range(NJ):
                pst = ps.tile([P, 512], F32, tag="mm")
                nc.tensor.matmul(pst, pf[:, j * P:(j + 1) * P],
                                 rw[:, t * 512:(t + 1) * 512], start=True, stop=True)
                nc.scalar.activation(Rf[:, j, t * 512:(t + 1) * 512], pst, AF.Copy)

        # ==================== S_B: fused Kf -> cmul -> KB per key chunk
        wkf = load_w("wkf", [P, NJ, D])
        gmat = load_w("gmat", [P, NJ, D])
        kn = persist.tile([P, NJ, TK2], BF, tag="kn")
        kfc = [None] * NT2

        def emit_kf(t):
            kfc[t] = kfcpool.tile([P, NJ, 512], BF, tag="kfc", name=f"kfc{t}")
            for j in range(NJ):
                pst = ps.tile([P, 512], F32, tag="mm")
                for i in range(NJ):
                    nc.tensor.matmul(pst, wkf[:, i, j * P:(j + 1) * P],
                                     kin[:, i, t * 512:(t + 1) * 512],
                                     start=(i == 0), stop=(i == NJ - 1))
                nc.scalar.activation(kfc[t][:, j, :], pst, AF.Identity,
                                     bias=bkf_sb[:, j:j + 1])
            # cmul in place: kf chunk becomes Z chunk (DVE)
            kf = kfc[t]
            rfs = Rf[:, :, t * 512:(t + 1) * 512]
            zf0 = cmt.tile([1, 512], BF, tag="zf0")
            zf4 = cmt.tile([1, 512], BF, tag="zf4")
            nc.vector.tensor_mul(zf0, kf[0:1, 0, :], rfs[0:1, 0, :])
            nc.vector.tensor_mul(zf4, kf[0:1, 4, :], rfs[0:1, 4, :])
            for j in range(4):
                t2 = cmt.tile([P, 512], BF, tag="cm_t2")
                t3 = cmt.tile([P, 512], BF, tag="cm_t3")
                nc.vector.tensor_mul(t2, kf[:, j + 4, :], rfs[:, j + 4, :])
                nc.vector.tensor_mul(t3, kf[:, j + 4, :], rfs[:, j, :])
                nc.vector.tensor_mul(kf[:, j + 4, :], kf[:, j, :], rfs[:, j + 4, :])
                nc.vector.tensor_tensor(kf[:, j + 4, :], kf[:, j + 4, :], t3, ALU.add)
                nc.vector.tensor_mul(kf[:, j, :], kf[:, j, :], rfs[:, j, :])
                nc.vector.tensor_tensor(kf[:, j, :], kf[:, j, :], t2, ALU.subtract)
            # rows 0 (re_0) and 512 (re_512) are purely real
            nc.vector.tensor_copy(kf[0:1, 0, :], zf0)
            nc.vector.tensor_copy(kf[0:1, 4, :], zf4)

        def emit_kb(t):
            psn = psrow.tile([1, 512], F32, tag="rowsum", name=f"psn_kb{t}")
            for j in range(NJ):
                pst = ps.tile([P, 512], F32, tag="mm")
                for i in range(NJ):
                    nc.tensor.matmul(pst, gmat[:, i, j * P:(j + 1) * P], kfc[t][:, i, :],
                                     start=(i == 0), stop=(i == NJ - 1))
                kb = kn[:, j, t * 512:(t + 1) * 512]
                nc.scalar.activation(kb, pst, AF.Copy)
                sq = tmp.tile([P, 512], BF, tag="sq")
                nc.vector.tensor_mul(sq, kb, kb)
                nc.tensor.matmul(psn, ones_k, sq, start=(j == 0), stop=(j == NJ - 1))
            # rk chunk = 1/sqrt(norm2) -> stage to DRAM for the column reload
            srow = rows.tile([1, 512], F32, tag="rowa", name="srow")
            nc.scalar.activation(srow, psn, AF.Sqrt)
            rrow = rows.tile([1, 512], F32, tag="rowb", name="rrow")
            nc.vector.reciprocal(rrow, srow)
            nc.gpsimd.dma_start(rk_stage[0:1, t * 512:(t + 1) * 512], rrow)

        emit_kf(0)
        emit_kf(1)
        emit_kb(0)
        emit_kb(1)
        # rk as [128,8] per-key-tile partition scalars, negated for the
        # (S*(-rk) + 1) epilogue
        rk_col = small.tile([P, NKT2], F32, tag="rk_col")
        nc.sync.dma_start(
            rk_col, rk_stage.rearrange("o (kt p) -> (o p) kt", p=P))
        nrk_col = small.tile([P, NKT2], F32, tag="nrk_col")
        nc.vector.tensor_scalar_mul(nrk_col, rk_col, -1.0)

        # ==================== S_F: Q proj (full TQ), tau, q_n
        wq = load_w("wq", [P, NJ, D])
        qn = persist.tile([P, NJ, TQ], BF, tag="big1", name="qn")
        rq_bc = small.tile([P, TQ], BF, tag="rq_bc")
        ssc = small.tile([1, NQT], F32, tag="ssc")
        for t in range(NQT):
            qin = qstream.tile([P, NJ, 512], BF, tag="qstream", name="qin")
            nc.sync.dma_start(qin, c3(din["qT"][:])[:, :, t * 512:(t + 1) * 512])
            psn = psrow.tile([1, 512], F32, tag="rowsum")
            for j in range(NJ):
                pst = ps.tile([P, 512], F32, tag="mm")
                for i in range(NJ):
                    nc.tensor.matmul(pst, wq[:, i, j * P:(j + 1) * P], qin[:, i, :],
                                     start=(i == 0), stop=(i == NJ - 1))
                qv = qn[:, j, t * 512:(t + 1) * 512]
                nc.scalar.activation(qv, pst, AF.Identity, bias=bq_sb[:, j:j + 1])
                sq = tmp.tile([P, 512], BF, tag="sq")
                nc.vector.tensor_mul(sq, qv, qv)
                nc.tensor.matmul(psn, ones_k, sq, start=(j == 0), stop=(j == NJ - 1))
            # sqrt of norms; accumulate sum(sqrt) for the surprise mean
            srow = rows.tile([1, 512], F32, tag="rowa", name="srow")
            nc.scalar.activation(srow, psn, AF.Sqrt, accum_out=ssc[0:1, t:t + 1])
            rrow = rows.tile([1, 512], F32, tag="rowb", name="rrow")
            nc.vector.reciprocal(rrow, srow)
            rrow16 = rows.tile([1, 512], BF, tag="rrow16")
            nc.vector.tensor_copy(rrow16, rrow)
            nc.gpsimd.dma_start(rq_stage[0:1, t * 512:(t + 1) * 512], rrow16)
        nc.sync.dma_start(
            rq_bc, bass.AP(tensor=rq_stage, offset=0, ap=[[0, P], [1, TQ]]))
        ss = small.tile([1, 1], F32, tag="ss")
        nc.vector.reduce_sum(ss, ssc, axis=mybir.AxisListType.X)
        # new_state = 0.95*astro + 0.05*ss/(32*2048)
        ns_t = small.tile([1, 1], F32, tag="ns_t")
        v1 = small.tile([1, 1], F32, tag="v1")
        nc.vector.tensor_scalar_mul(v1, ss, (1.0 - ASTRO_DECAY) / (32.0 * TQ))
        v2 = small.tile([1, 1], F32, tag="v2")
        nc.vector.tensor_scalar_mul(v2, scal_sb[0:1, 0:1], ASTRO_DECAY)
        nc.vector.tensor_add(ns_t, v1, v2)
        nc.gpsimd.dma_start(ns_out[:], ns_t)
        # c = tau/4 = max(1 + astro_scale*ns, 0.001)/4
        c_t = small.tile([1, 1], F32, tag="c_t")
        nc.vector.tensor_mul(c_t, ns_t, scal_sb[0:1, 1:2])
        nc.vector.tensor_scalar(c_t, c_t, 1.0, 0.001, ALU.add, ALU.max)
        nc.vector.tensor_scalar_mul(c_t, c_t, 0.25 * TAU_BASE)
        nc.gpsimd.dma_start(c_stage[:], c_t)
        c_sb = small.tile([P, 1], F32, tag="c_sb")
        nc.sync.dma_start(
            c_sb, bass.AP(tensor=c_stage, offset=0, ap=[[0, P], [1, 1]]))
        for j in range(NJ):
            nc.vector.tensor_mul(qn[:, j, :], qn[:, j, :], rq_bc)

        # ==================== S_E: VW = vT.T @ (WvWo), SBUF resident
        vwo = load_w("vwo", [P, NJ, D])
        VW = kinvw.tile([P, NKT2, D], BF, tag="kinvw", name="VW")
        for to in range(NKT2):
            vin = attst.tile([P, NJ, P], BF, tag="vin", name="vin")
            nc.sync.dma_start(vin, c3(din["vT"][:])[:, :, to * P:(to + 1) * P])
            for n in range(2):
                pst = ps.tile([P, 512], F32, tag="mm")
                for i in range(NJ):
                    nc.tensor.matmul(pst, vin[:, i, :], vwo[:, i, n * 512:(n + 1) * 512],
                                     start=(i == 0), stop=(i == NJ - 1))
                nc.scalar.activation(VW[:, to, n * 512:(n + 1) * 512], pst, AF.Copy)

        # ==================== S_H/S_I: scores + partial out per query chunk
        for qc in range(NQT):
            scT = scorepool.tile([P, NKT2, 512], BF, tag="scT", name=f"scT{qc}")
            rs_ps = psrow.tile([1, 512], F32, tag="rowsum", name=f"rs_ps{qc}")
            for kt in range(NKT2):
                ps_s = ps.tile([P, 512], F32, tag="mm")
                for j in range(NJ):
                    nc.tensor.matmul(ps_s, kn[:, j, kt * P:(kt + 1) * P],
                                     qn[:, j, qc * 512:(qc + 1) * 512],
                                     start=(j == 0), stop=(j == NJ - 1))
                u0 = tmp.tile([P, 512], BF, tag="u0")
                # u0 = 1 - cos = S*(-rk[key]) + 1
                nc.vector.tensor_scalar(u0, ps_s, nrk_col[:, kt:kt + 1], 1.0,
                                        ALU.mult, ALU.add)
                u1 = tmp.tile([P, 512], BF, tag="u1")
                nc.scalar.activation(u1, u0, AF.Square)
                u2 = tmp.tile([P, 512], BF, tag="u2")
                nc.vector.tensor_scalar_mul(u2, u1, c_sb)
                # scores = relu(1 - c*(1-cos)^2)
                nc.scalar.activation(scT[:, kt, :], u2, AF.Relu,
                                     bias=1.0, scale=-1.0)
                nc.tensor.matmul(rs_ps, ones_k, scT[:, kt, :],
                                 start=(kt == 0), stop=(kt == NKT2 - 1))
            # stage and combine this chunk's partial row sums right away
            rs_row = rows.tile([1, 512], F32, tag="rowa", name="rs_row")
            nc.vector.tensor_copy(rs_row, rs_ps)
            nc.gpsimd.dma_start(rs_in[qc][:], rs_row)
            nc.gpsimd.collective_compute(
                "AllReduce", ALU.add, replica_groups=CC_GROUPS,
                ins=[rs_in[qc][:]], outs=[rs_out[qc][:]])
            # partial out = VW.T @ scT for this chunk
            for j2g in range(2):
                pcs = [ps.tile([P, 512], F32, tag="mm", name=f"pcs{i}") for i in range(4)]
                for kt in range(NKT2):
                    for jj in range(4):
                        nc.tensor.matmul(pcs[jj],
                                         VW[:, kt, j2g * 512 + jj * P:j2g * 512 + (jj + 1) * P],
                                         scT[:, kt, :],
                                         start=(kt == 0), stop=(kt == NKT2 - 1))
                for jj in range(4):
                    j = j2g * 4 + jj
                    o_t = outstage.tile([P, 512], BF, tag="outstage")
                    nc.scalar.activation(o_t, pcs[jj], AF.Copy)
                    nc.gpsimd.dma_start(po_in[qc][j * P:(j + 1) * P, :], o_t)
            # pair AllReduce for this chunk's partial out (pipelined)
            nc.gpsimd.collective_compute(
                "AllReduce", ALU.add, replica_groups=CC_GROUPS,
                ins=[po_in[qc][:]], outs=[po_out[qc][:]])
            # finalize this chunk (overlaps later chunks' compute)
            rs_sl = rows.tile([1, 512], F32, tag="rowa", name="rs_sl")
            nc.sync.dma_start(rs_sl, rs_out[qc][:])
            rinv = rows.tile([1, 512], F32, tag="rowb", name="rinv")
            nc.vector.reciprocal(rinv, rs_sl)
            nc.gpsimd.dma_start(rsf_stage[0:1, qc * 512:(qc + 1) * 512], rinv)
            rsf_bc = rsfpool.tile([P, 512], F32, tag="rsf_bc", name=f"rsf_bc{qc}")
            nc.sync.dma_start(
                rsf_bc, bass.AP(tensor=rsf_stage, offset=qc * 512,
                                ap=[[0, P], [1, 512]]))
            for j in range(NJ):
                po_sl = attst.tile([P, 512], BF, tag="po_sl", name="po_sl")
                nc.sync.dma_start(po_sl, po_out[qc][j * P:(j + 1) * P, :])
                o_t = tmp.tile([P, 512], F32, tag="o_t")
                nc.vector.tensor_mul(o_t, po_sl, rsf_bc)
                nc.vector.tensor_scalar_add(o_t, o_t, bvo_sb[:, j:j + 1])
                nc.gpsimd.dma_start(
                    outT[j * P:(j + 1) * P, qc * 512:(qc + 1) * 512], o_t)

    return nc


_CACHE = {}


def _get_nc():
    if "nc" not in _CACHE:
        nc = bacc.Bacc(None, target_bir_lowering=False)
        _emit(nc)
        nc.finalize()
        _CACHE["nc"] = nc
    return _CACHE["nc"]


def build_in_maps(inputs):
    """Host-side prep: foldings, packing, per-core sharding."""
    F, G = _build_dft_mats(D)
    role = np.asarray(inputs["role_matrix"], np.float32)
    role = role / np.clip(np.linalg.norm(role, axis=-1, keepdims=True), 1e-12, None)
    PF = _pack_rfft(role)

    f32 = lambda x: np.asarray(x, np.float32)
    bf = lambda x: np.ascontiguousarray(f32(x)).astype(BF16)
    btile = lambda x: np.ascontiguousarray(f32(x).reshape(-1, P).T.copy())

    Wk, Wv, Wo, Wr1 = f32(inputs["Wk"]), f32(inputs["Wv"]), f32(inputs["Wo"]), f32(inputs["Wr1"])
    bk, bv, bo, br1 = f32(inputs["bk"]), f32(inputs["bv"]), f32(inputs["bo"]), f32(inputs["br1"])

    weights = {
        "wq": bf(inputs["Wq"]),
        "wkf": bf(Wk @ F),
        "gmat": bf(G),
        "vwo": bf(Wv @ Wo),
        "wkr1": bf(Wk @ Wr1),
        "wr2": bf(inputs["Wr2"]),
        "pf": bf(PF),
        "bq": btile(inputs["bq"]),
        "bkf": btile(bk @ F),
        "bvo": btile(bv @ Wo + bo),
        "brk": btile(bk @ Wr1 + br1),
    }

    in_maps = []
    for core in range(8):
        b, h = core // 2, core % 2
        m = dict(weights)
        m["qT"] = np.ascontiguousarray(f32(inputs["q_in"][b]).T).astype(BF16)
        m["kT"] = np.ascontiguousarray(
            f32(inputs["k_in"][b])[h * TK2:(h + 1) * TK2].T).astype(BF16)
        m["vT"] = np.ascontiguousarray(
            f32(inputs["v_in"][b])[h * TK2:(h + 1) * TK2].T).astype(BF16)
        m["scal"] = np.array(
            [[np.float32(inputs["astrocyte_state"][b]),
              np.float32(np.asarray(inputs["astro_scale"]).reshape(-1)[0])]],
            np.float32)
        in_maps.append(m)
    return in_maps


def kernel(q_in, k_in, v_in, astrocyte_state, Wq, bq, Wk, bk, Wv, bv, Wo, bo,
           role_matrix, Wr1, br1, Wr2, astro_scale, **_ignored):
    nc = _get_nc()
    inputs = dict(q_in=q_in, k_in=k_in, v_in=v_in, astrocyte_state=astrocyte_state,
                  Wq=Wq, bq=bq, Wk=Wk, bk=bk, Wv=Wv, bv=bv, Wo=Wo, bo=bo,
                  role_matrix=role_matrix, Wr1=Wr1, br1=br1, Wr2=Wr2,
                  astro_scale=astro_scale)
    in_maps = build_in_maps(inputs)
    res = run_bass_kernel_spmd(nc, in_maps, core_ids=list(range(8)))

    output = np.empty((B, TQ, D), np.float32)
    new_state = np.empty((B,), np.float32)
    for b in range(B):
        output[b] = res.results[2 * b]["outT"].T
        new_state[b] = res.results[2 * b]["ns_out"][0, 0]
    return output, new_state
